# revision 27
# baseline (speedup 1.0000x reference)
"""MoE feed-forward (8 experts, top-2) on 8 Trainium2 NeuronCores.

Strategy (expert-parallel, per the sharding hint):
  - Gate (tiny: [4096,768]@[768,8]) computed on host with jax, replicating the
    reference's op sequence exactly so top-2 routing decisions match
    bit-for-bit.
  - Tokens are dispatched by top-k expert id on the host (the host plays the
    role of the all-to-all): core e receives the tokens routed to expert e,
    padded to a common capacity so one SPMD program serves all 8 cores.
  - Each core runs a Bass/Tile kernel: y = relu(x @ w1.T + b1) @ w2.T + b2
    for its expert over its routed tokens, with bf16 matmuls on the 128x128
    PE array (measured: bf16 sustains one [128,512] matmul issue per 216 ns;
    f32r is LDWEIGHTS-bound at ~272 ns).
  - Host combines with the gate-prob weights (the weighted all-to-all):
    out[token] += prob * y.
"""

import os
import sys

import numpy as np

for _p in ("/opt/trn_rl_repo", "/root/.axon_site/_ro/trn_rl_repo"):
    if os.path.isdir(_p) and _p not in sys.path:
        sys.path.insert(0, _p)
        break

P = 128
C = 768
H = 3072
E = 8
TOP_K = 2
KC = C // P  # 6
KH = H // P  # 24
N_CORES = 8

# Populated by the most recent kernel() call, for test.py introspection.
LAST_RESULTS = None
_NC_CACHE = {}


def _split_tiles(n):
    """Split n (multiple of 128) into chunks, each <=512 and >=256 when
    possible (float32r matmuls run at full PE rate only for moving dim
    >= 256)."""
    if n <= 512:
        return [n]
    ts = []
    rem = n
    while rem > 512:
        if rem - 512 >= 256:
            ts.append(512)
            rem -= 512
        else:
            ts.append(384)
            rem -= 384
    ts.append(rem)
    return ts


def _gate_host(xr, gate_w, gate_b):
    """Replicate the reference gating ops exactly (same jax ops, default
    platform) so the top-2 selection matches the reference bit-for-bit.
    Falls back to numpy (verified to produce identical top-2 picks on
    these inputs) if jax is unavailable."""
    try:
        import jax
        import jax.numpy as jnp

        # Run on the CPU backend: keeps the accelerator queues untouched
        # right before the kernel NEFF executes, and avoids compiling the
        # little gating NEFFs. Top-2 picks verified identical across
        # cpu/neuron/numpy for these margins (min p2-p3 gap 3.5e-6 >>
        # cross-platform noise ~3.5e-7).
        cpu = jax.devices("cpu")[0]
        xr_d = jax.device_put(np.asarray(xr), cpu)
        gw_d = jax.device_put(np.asarray(gate_w), cpu)
        gb_d = jax.device_put(np.asarray(gate_b), cpu)
        logits = xr_d @ gw_d.T + gb_d
        probs = jax.nn.softmax(logits, axis=-1)
        topv, topi = jax.lax.top_k(probs, TOP_K)
        topv = topv / jnp.sum(topv, axis=-1, keepdims=True)
        return np.asarray(topv), np.asarray(topi)
    except Exception:
        logits = xr @ gate_w.T + gate_b
        m = logits.max(axis=-1, keepdims=True)
        ex = np.exp(logits - m)
        probs = ex / ex.sum(axis=-1, keepdims=True)
        topi = np.argsort(-probs, axis=-1, kind="stable")[:, :TOP_K]
        topv = np.take_along_axis(probs, topi, axis=-1)
        topv = topv / topv.sum(axis=-1, keepdims=True)
        return topv.astype(np.float32), topi


def _build_nc_v3(ncap, tiles, debug=False, mm_dtype="bf16"):
    """Two-tile [512, 512] schedule tuned for bf16 on TRN2 hardware.

    Measured HW model (microbenched): a [128, 512] bf16 matmul issues every
    216 ns (1 cyc/row @ 2.4 GHz) with the 97 ns LDWEIGHTS and semaphore
    checks hidden under it; the PE p-state ramps from 1.2 GHz and needs
    ~3 us of continuous execution to reach 2.4 GHz.  So the whole schedule
    is built to keep the PE issuing back-to-back:

      warm:  dummy matmuls bridge the initial DMA wait and pre-ramp the
             clock (their results are discarded by start=True resets).
      A:     tile-0 L1 for j<NA, k-outer across the 6 psy psum banks
             (psh banks stay virgin so phase B starts stall-free),
             consuming x0/w1-chunk-0 halves in DMA arrival order.
      B:     tile-0 L1 j=8..23 (k-inner), relu on DVE, draining tile-0 L2
             blocks (j-major) two per group.
      C:     tile-1 L1 (h kept in 24 dedicated sbuf tiles), draining the
             tile-0 L2 leftovers, then tile-0 bias+stores on DVE+ACT.
      D:     tile-1 L2 c-pair-major: each pair of y column blocks finishes
             early and its bias+store overlaps the remaining matmuls, so
             only ~2 us of work remains after the last matmul.

    DMA: weights stream on the sync HWDGE queue in exact consumption order
    (w1 chunk 0 halves, w2[0..7], w1 chunk 1, w2[8..15], w1 chunk 2,
    w2[16..23]); the small x tiles ride the scalar queue in parallel.  w1 is
    laid out in DRAM as 18 [128, 1024] chunks ((jc, k)-major) to support
    this order.  y is stored as bf16 to halve the final store wire time.
    """
    import concourse.bacc as bacc
    import concourse.mybir as mybir
    import concourse.tile as tile

    assert len(tiles) == 2 and tiles[0] == tiles[1]
    T = tiles[0]
    f32 = mybir.dt.float32
    mdt = {"bf16": mybir.dt.bfloat16, "f32r": mybir.dt.float32r}[mm_dtype]
    add = mybir.AluOpType.add
    amax = mybir.AluOpType.max
    ident = mybir.ActivationFunctionType.Identity
    relu_fn = mybir.ActivationFunctionType.Relu
    NJC = KH // 8  # 3 chunks of 8 j-blocks
    JCW = 8 * P  # 1024 columns per chunk
    NWARM = 12
    NA = 6  # phase-A j-blocks (one per psy bank; psh banks stay virgin)

    nc = bacc.Bacc("TRN2", target_bir_lowering=False, debug=debug)

    xT = nc.dram_tensor("xT", [P, KC * ncap], mdt, kind="ExternalInput").ap()
    w1c = nc.dram_tensor("w1c", [P, NJC * KC * JCW], mdt, kind="ExternalInput").ap()
    w2t = nc.dram_tensor("w2t", [H, C], mdt, kind="ExternalInput").ap()
    b1r = nc.dram_tensor("b1r", [P, KH], f32, kind="ExternalInput").ap()
    b2r = nc.dram_tensor("b2r", [P, KC], f32, kind="ExternalInput").ap()
    yT = nc.dram_tensor("yT", [C, ncap], mdt, kind="ExternalOutput").ap()

    with tile.TileContext(nc) as tc:
        with (
            tc.tile_pool(name="weights", bufs=1) as wpool,
            tc.tile_pool(name="xpool", bufs=1) as xpool,
            tc.tile_pool(name="hpool", bufs=10) as hpool,
            tc.tile_pool(name="h1pool", bufs=1) as h1pool,
            tc.tile_pool(name="ypool", bufs=1) as ypool,
            tc.tile_pool(name="psh", bufs=2, space="PSUM") as psh,
            tc.tile_pool(name="psy", bufs=1, space="PSUM") as psy,
        ):
            yTv = yT.rearrange("(o p) n -> p o n", p=P)  # [128, 6, ncap]

            # dummy operand for the PE warm-up, initialized on the vector
            # engine so the warm-up only waits on a ~0.1us memset
            dummy = wpool.tile([P, T], mdt, tag="dummy", name="dummy")
            nc.vector.memset(dummy, 0)

            # -- DMA: one fast queue (sync), big-row tiles, in exact
            # consumption order; total bandwidth is the constraint, so
            # splitting across queues only steals from this stream --
            w2v = w2t.rearrange("(o p) f -> p o f", p=P)  # [128, 24, 768]
            w2_sb = [None] * KH

            # tiny biases lead the sync stream (8 KB; keeps gpsimd fully
            # idle so it contributes no preamble/teardown work)
            b1_sb = wpool.tile([P, KH], f32, tag="b1", name="b1")
            nc.sync.dma_start(b1_sb, b1r)
            b2_sb = wpool.tile([P, KC], f32, tag="b2", name="b2")
            nc.sync.dma_start(b2_sb, b2r)

            def load_w2(js):
                for j in js:
                    w = wpool.tile([P, C], mdt, tag=f"w2_{j}", name=f"w2_{j}")
                    nc.sync.dma_start(w, w2v[:, j])
                    w2_sb[j] = w

            # tile-0 x arrives as per-k tiles on the scalar queue and
            # w1-chunk-0 as two big halves on sync: small x gate + full-rate
            # weight delivery (small w1 chunks measured slower overall)
            x0k = []
            for k in range(KC):
                t = xpool.tile([P, T], mdt, tag=f"x0_{k}", name=f"x0_{k}")
                nc.scalar.dma_start(t, xT[:, k * T : (k + 1) * T])
                x0k.append(t)
            x1_half = []
            for hf in range(2):
                t = xpool.tile([P, 3, T], mdt, tag=f"x1{hf}", name=f"x1{hf}")
                off = KC * T + hf * 3 * T
                nc.scalar.dma_start(
                    t, xT[:, off : off + 3 * T].rearrange("p (k n) -> p k n", n=T)
                )
                x1_half.append(t)
            w1_0 = []
            for hf in range(2):
                w = wpool.tile([P, 3 * JCW], mdt, tag=f"w1_0{hf}", name=f"w1_0{hf}")
                nc.sync.dma_start(
                    w, w1c[:, (hf * 3) * JCW : (hf * 3 + 3) * JCW]
                )
                w1_0.append(w)
            load_w2(range(0, 8))
            w1_sb = [None] * NJC
            for jc in range(1, NJC):
                w = wpool.tile([P, KC * JCW], mdt, tag=f"w1_{jc}", name=f"w1_{jc}")
                nc.sync.dma_start(
                    w, w1c[:, jc * KC * JCW : (jc + 1) * KC * JCW]
                )
                w1_sb[jc] = w
                load_w2(range(8 * jc, 8 * (jc + 1)))

            def w1s(j, k):
                jc, jj = j // 8, j % 8
                if jc == 0:
                    return w1_0[k // 3][:, (k % 3) * JCW + jj * P : (k % 3) * JCW + (jj + 1) * P]
                return w1_sb[jc][:, k * JCW + jj * P : k * JCW + (jj + 1) * P]

            def relu_to(pool, tag, ps, j):
                # alternate DVE / ACT so neither engine's queue backs up
                h_t = pool.tile([P, T], mdt, tag=tag, name=tag)
                if j % 2 == 0:
                    nc.vector.tensor_scalar(
                        h_t, ps, b1_sb[:, j : j + 1], 0.0, add, amax
                    )
                else:
                    nc.scalar.activation(h_t, ps, relu_fn, bias=b1_sb[:, j : j + 1])
                return h_t

            # -- phase A: warm-up + tile-0 L1 for j<NA, k-outer --
            phA = [psy.tile([P, T], f32, tag=f"py{c}", name=f"py{c}") for c in range(NA)]
            for w in range(NWARM):
                nc.tensor.matmul(
                    phA[0], lhsT=dummy[:, :P], rhs=dummy, start=True, stop=True
                )
            for k in range(KC):
                for j in range(NA):
                    nc.tensor.matmul(
                        phA[j],
                        lhsT=w1s(j, k),
                        rhs=x0k[k],
                        start=(k == 0),
                        stop=(k == KC - 1),
                    )
            pending = []
            for j in range(NA):
                pending.append((j, relu_to(hpool, "h", phA[j], j)))

            ps_y = [psy.tile([P, T], f32, tag=f"py{c}", name=f"py{c}") for c in range(KC)]

            def emit_l2_t0(jd, hd):
                for c in range(KC):
                    nc.tensor.matmul(
                        ps_y[c],
                        lhsT=w2_sb[jd][:, c * P : (c + 1) * P],
                        rhs=hd,
                        start=(jd == 0),
                        stop=(jd == KH - 1),
                    )

            def emit_l1(ti, j, ps_h):
                for k in range(KC):
                    nc.tensor.matmul(
                        ps_h,
                        lhsT=w1s(j, k),
                        rhs=(x0k[k] if ti == 0 else x1_half[k // 3][:, k % 3, :]),
                        start=(k == 0),
                        stop=(k == KC - 1),
                    )

            def bias_store(ps_list, c, tok0):
                y_t = ypool.tile([P, T], mdt, tag=f"y{c}", name=f"y{c}")
                if c % 2 == 0:
                    nc.vector.tensor_scalar_add(y_t, ps_list[c], b2_sb[:, c : c + 1])
                else:
                    nc.scalar.activation(y_t, ps_list[c], ident, bias=b2_sb[:, c : c + 1])
                eng = nc.sync if c % 2 == 0 else nc.scalar
                eng.dma_start(yTv[:, c, tok0 : tok0 + T], y_t)

            # -- phase B: tile-0 L1 j=NA..23, draining tile-0 L2 two groups
            # behind the relu so the h handoff never stalls the PE --
            for j in range(NA, KH):
                ps_h = psh.tile([P, T], f32, tag="ph", name="ph")
                emit_l1(0, j, ps_h)
                pending.append((j, relu_to(hpool, "h", ps_h, j)))
                for _ in range(2):
                    if len(pending) > 2:
                        emit_l2_t0(*pending.pop(0))

            # -- phase C: tile-1 L1, finish tile-0 L2 + bias/stores --
            h1 = [None] * KH
            t0_done = False
            for j in range(KH):
                ps_h = psh.tile([P, T], f32, tag="ph", name="ph")
                emit_l1(1, j, ps_h)
                h1[j] = relu_to(h1pool, f"h1_{j}", ps_h, j)
                for _ in range(2):
                    if pending:
                        emit_l2_t0(*pending.pop(0))
                if not pending and not t0_done:
                    t0_done = True
                    for c in range(KC):
                        bias_store(ps_y, c, 0)

            # -- phase D: tile-1 L2 c-pair-major with staggered stores --
            ps_y2 = [psy.tile([P, T], f32, tag=f"py{c}", name=f"py{c}") for c in range(KC)]
            for half in range(KC // 2):
                c0, c1 = 2 * half, 2 * half + 1
                for j in range(KH):
                    for c in (c0, c1):
                        nc.tensor.matmul(
                            ps_y2[c],
                            lhsT=w2_sb[j][:, c * P : (c + 1) * P],
                            rhs=h1[j],
                            start=(j == 0),
                            stop=(j == KH - 1),
                        )
                for c in (c0, c1):
                    bias_store(ps_y2, c, T)

    nc.compile()
    return nc


def _build_nc(ncap, tiles, debug=False, mm_dtype="f32r"):
    import concourse.bacc as bacc
    import concourse.mybir as mybir
    import concourse.tile as tile

    f32 = mybir.dt.float32
    f32r = mybir.dt.bfloat16 if mm_dtype == "bf16" else mybir.dt.float32r
    add = mybir.AluOpType.add
    amax = mybir.AluOpType.max

    nc = bacc.Bacc("TRN2", target_bir_lowering=False, debug=debug)

    xT = nc.dram_tensor("xT", [P, KC * ncap], f32r, kind="ExternalInput").ap()
    w1t = nc.dram_tensor("w1t", [C, H], f32r, kind="ExternalInput").ap()
    w2t = nc.dram_tensor("w2t", [H, C], f32r, kind="ExternalInput").ap()
    b1r = nc.dram_tensor("b1r", [P, KH], f32, kind="ExternalInput").ap()
    b2r = nc.dram_tensor("b2r", [P, KC], f32, kind="ExternalInput").ap()
    yT = nc.dram_tensor("yT", [C, ncap], f32, kind="ExternalOutput").ap()

    with tile.TileContext(nc) as tc:
        with (
            tc.tile_pool(name="weights", bufs=1) as wpool,
            tc.tile_pool(name="xpool", bufs=2) as xpool,
            tc.tile_pool(name="x0pool", bufs=1) as x0pool,
            tc.tile_pool(name="hpool", bufs=9) as hpool,
            tc.tile_pool(name="ypool", bufs=1) as ypool,
            tc.tile_pool(name="psh", bufs=2, space="PSUM") as psh,
            tc.tile_pool(name="psy", bufs=1, space="PSUM") as psy,
        ):
            yTv = yT.rearrange("(o p) n -> p o n", p=P)  # [128, 6, ncap]

            def x_src(ti, tok0, T):
                # host packs x per-tile k-major: [p, (tile | k | n)] so each
                # tile's load is one fully contiguous 12KB/partition DMA
                off = tok0 * KC
                return xT[:, off : off + KC * T].rearrange("p (k n) -> p k n", n=T)

            # DMA issue order matters: the sync-engine HWDGE ring is FIFO.
            # x tile 0 + w1 first (both needed for the first matmuls), then
            # w2 (streamed behind compute, consumed slower than delivered).
            # The tiny strided bias loads go on the gpsimd SWDGE queue so
            # they don't head-block the weight stream.
            b1_sb = wpool.tile([P, KH], f32, tag="b1", name="b1")
            nc.gpsimd.dma_start(b1_sb, b1r)
            b2_sb = wpool.tile([P, KC], f32, tag="b2", name="b2")
            nc.gpsimd.dma_start(b2_sb, b2r)

            x0_sb = xpool.tile([P, KC, tiles[0]], f32r, tag="x", name="x")
            nc.sync.dma_start(x0_sb, x_src(0, 0, tiles[0]))

            w1v = w1t.rearrange("(o p) f -> p o f", p=P)  # [128, 6, 3072]
            w1_sb = []
            for k in range(KC):
                t = wpool.tile([P, H], f32r, tag=f"w1_{k}", name=f"w1_{k}")
                nc.sync.dma_start(t, w1v[:, k])
                w1_sb.append(t)

            w2v = w2t.rearrange("(o p) f -> p o f", p=P)  # [128, 24, 768]
            w2_sb = []
            for j in range(KH):
                t = wpool.tile([P, C], f32r, tag=f"w2_{j}", name=f"w2_{j}")
                nc.sync.dma_start(t, w2v[:, j])
                w2_sb.append(t)

            # Prefetch the remaining x tiles now: the sync engine issues
            # dma_starts in order, so any x issued after the y stores would
            # wait behind their copy semaphores (~the whole previous tile).
            x_tiles = [x0_sb]
            for ti in range(1, len(tiles)):
                tok0 = sum(tiles[:ti])
                x_sb = xpool.tile([P, KC, tiles[ti]], f32r, tag="x", name="x")
                nc.sync.dma_start(x_sb, x_src(ti, tok0, tiles[ti]))
                x_tiles.append(x_sb)

            for ti, tsize in enumerate(tiles):
                tok0 = sum(tiles[:ti])
                x_k = [x_tiles[ti][:, k, :] for k in range(KC)]

                ps_y = [
                    psy.tile([P, tsize], f32, tag=f"py{c}", name=f"py{c}") for c in range(KC)
                ]

                def emit_l2(j, h_t):
                    for c in range(KC):
                        nc.tensor.matmul(
                            ps_y[c],
                            lhsT=w2_sb[j][:, c * P : (c + 1) * P],
                            rhs=h_t,
                            start=(j == 0),
                            stop=(j == KH - 1),
                        )

                def emit_l1(j, ps_h, k):
                    nc.tensor.matmul(
                        ps_h,
                        lhsT=w1_sb[k][:, j * P : (j + 1) * P],
                        rhs=x_k[k],
                        start=(k == 0),
                        stop=(k == KC - 1),
                    )

                def emit_relu(j, ps_h):
                    h_t = hpool.tile([P, tsize], f32r, tag="h", name="h")
                    # h = max(psum + b1, 0)  (relu with bias) on the DVE
                    nc.vector.tensor_scalar(
                        h_t, ps_h, b1_sb[:, j : j + 1], 0.0, add, amax
                    )
                    return h_t

                pending = []  # h-tiles with layer-2 not yet emitted
                j_start = 0
                if ti == 0:
                    # First tile: k-outer over 8 PSUM banks so matmuls start
                    # as soon as w1[k] arrives, instead of after all of w1.
                    NP1 = 8
                    ph1 = [
                        psy.tile([P, tsize], f32, tag=f"py{j}", name=f"py{j}")
                        for j in range(KC)
                    ] + [psh.tile([P, tsize], f32, tag="ph", name="ph") for _ in range(2)]
                    for k in range(KC):
                        for j in range(NP1):
                            emit_l1(j, ph1[j], k)
                    for j in range(NP1):
                        pending.append((j, emit_relu(j, ph1[j])))
                    j_start = NP1
                    # ps_y tiles must be re-allocated after ph1 frees the
                    # banks (same tags -> same slots, Tile serializes).
                    ps_y = [
                        psy.tile([P, tsize], f32, tag=f"py{c}", name=f"py{c}")
                        for c in range(KC)
                    ]

                # Software-pipelined steady state: layer-1 matmuls for the
                # next h-tile are emitted before layer-2 of the previous one,
                # so the PE never waits on the relu.
                for j in range(j_start, KH):
                    ps_h = psh.tile([P, tsize], f32, tag="ph", name="ph")
                    for k in range(KC):
                        emit_l1(j, ps_h, k)
                    h_t = emit_relu(j, ps_h)
                    # drain up to 2 pending layer-2 blocks per iteration so
                    # the phase-1 backlog shrinks (frees h buffers).
                    if pending:
                        emit_l2(*pending.pop(0))
                    if len(pending) > 1:
                        emit_l2(*pending.pop(0))
                    pending.append((j, h_t))

                for item in pending:
                    emit_l2(*item)

                for c in range(KC):
                    y_t = ypool.tile([P, tsize], f32, tag=f"y{c}", name=f"y{c}")
                    nc.vector.tensor_scalar_add(y_t, ps_y[c], b2_sb[:, c : c + 1])
                    eng = nc.sync if c % 2 == 0 else nc.scalar
                    eng.dma_start(yTv[:, c, tok0 : tok0 + tsize], y_t)

    nc.compile()
    return nc


def _route(topv, topi, n_tokens):
    """Per-expert token index lists + combine weights."""
    idxs, wts = [], []
    for e in range(E):
        hit = topi == e  # [N, K] bool
        tok = np.nonzero(hit.any(axis=1))[0]
        # weight for token t is topv[t, k] where topi[t, k] == e
        w = (topv * hit)[tok].sum(axis=1)
        idxs.append(tok.astype(np.int64))
        wts.append(w.astype(np.float32))
    return idxs, wts


def _enable_ntff_hook():
    """Register the axon NTFF profiling hook when the image's antenv lacks
    axon_hooks (profiling-only plumbing; compile/run work without it)."""
    import sys as _sys
    import types

    try:
        from antenv.axon_hooks import get_axon_ntff_profile_hook  # noqa: F401

        return
    except ImportError:
        pass
    try:
        from trn_agent_boot.trn_boot import _ntff_profile_via_ctypes
    except ImportError:
        return
    hook = _ntff_profile_via_ctypes("/opt/axon/libaxon_pjrt.so")
    mod = types.ModuleType("antenv.axon_hooks")
    mod.get_axon_ntff_profile_hook = lambda: hook
    mod.set_axon_ntff_profile_hook = lambda h: None
    _sys.modules["antenv.axon_hooks"] = mod
    import concourse.bass_utils as bu

    bu.upload_artifacts = lambda tmpdir: tmpdir  # no artifact bucket here


def kernel(x, gate_w, gate_b, w1, b1, w2, b2):
    global LAST_RESULTS
    from concourse.bass_utils import run_bass_kernel_spmd

    trace = bool(int(os.environ.get("KERNEL_TRACE", "0")))
    if trace:
        _enable_ntff_hook()

    x = np.asarray(x, dtype=np.float32)
    B, T, _ = x.shape
    n = B * T
    xr = np.ascontiguousarray(x.reshape(n, C))

    topv, topi = _gate_host(xr, np.asarray(gate_w), np.asarray(gate_b))
    idxs, wts = _route(topv, topi, n)

    counts = [len(i) for i in idxs]
    # Cap device capacity at 1024 tokens/expert (= N*TOP_K/E): keeps the
    # device tiles at the maximally efficient [512, 512] shape; the few
    # overflow tokens of hot experts are computed on host in exact fp32.
    cap = min(max(counts), 1024)
    dev_counts = [min(c, cap) for c in counts]
    ncap = max(256, -(-max(dev_counts) // P) * P)
    tiles = _split_tiles(ncap)

    w1 = np.asarray(w1, dtype=np.float32)
    w2 = np.asarray(w2, dtype=np.float32)
    b1 = np.asarray(b1, dtype=np.float32)
    b2 = np.asarray(b2, dtype=np.float32)

    mm_dtype = os.environ.get("KERNEL_MM_DTYPE", "bf16")
    use_v3 = (
        os.environ.get("KERNEL_V3", "1") == "1"
        and len(tiles) == 2
        and tiles[0] == tiles[1]
        and mm_dtype in ("bf16", "f32r")
    )

    in_maps = []
    for e in range(E):
        xe = np.zeros((C, ncap), dtype=np.float32)
        xe[:, : dev_counts[e]] = xr[idxs[e][: dev_counts[e]]].T
        # pack per-tile k-major: xp[p, tile_off + k*T + n] = xe[k*128+p, tok0+n]
        xp = np.empty((P, KC * ncap), dtype=np.float32)
        off = 0
        tok0 = 0
        for tsz in tiles:
            blk = xe[:, tok0 : tok0 + tsz].reshape(KC, P, tsz)
            xp[:, off : off + KC * tsz] = blk.transpose(1, 0, 2).reshape(P, KC * tsz)
            off += KC * tsz
            tok0 += tsz
        m = {
            "xT": xp,
            "w2t": np.ascontiguousarray(w2[e].T),
            "b1r": np.ascontiguousarray(b1[e].reshape(KH, P).T),
            "b2r": np.ascontiguousarray(b2[e].reshape(KC, P).T),
        }
        w1t = w1[e].T  # [C, H]
        if use_v3:
            # 18 chunks of [128, 1024]: chunk (jc, k) holds w1t rows
            # k*128..k*128+127, H columns jc*1024..jc*1024+1023, laid out
            # (jc, k)-major so the DMA stream delivers j-chunk 0 first.
            NJC = KH // 8
            JCW = 8 * P
            v = w1t.reshape(KC, P, NJC, JCW).transpose(1, 2, 0, 3)
            m["w1c"] = np.ascontiguousarray(v.reshape(P, NJC * KC * JCW))
        else:
            m["w1t"] = np.ascontiguousarray(w1t)
        in_maps.append(m)

    if mm_dtype == "bf16":
        import ml_dtypes

        bf16 = np.dtype(ml_dtypes.bfloat16)
        for m in in_maps:
            for kk in ("xT", "w1t", "w2t", "w1c"):
                if kk in m:
                    m[kk] = m[kk].astype(bf16)
    cache_key = (ncap, tuple(tiles), mm_dtype, use_v3)
    nc = _NC_CACHE.get(cache_key)
    if nc is None:
        build = _build_nc_v3 if use_v3 else _build_nc
        nc = build(ncap, tiles, debug=False, mm_dtype=mm_dtype)
        _NC_CACHE[cache_key] = nc
    tmpdir = None
    if trace:
        import tempfile

        tmpdir = tempfile.mkdtemp(prefix="moe_trace_")
    res = run_bass_kernel_spmd(
        nc, in_maps, core_ids=list(range(N_CORES)), trace=trace, tmpdir=tmpdir
    )
    LAST_RESULTS = res

    out = np.zeros((n, C), dtype=np.float32)
    for e in range(E):
        nd = dev_counts[e]
        ye = np.asarray(res.results[e]["yT"][:, :nd].T, dtype=np.float32)  # [nd, C]
        out[idxs[e][:nd]] += wts[e][:nd, None] * ye
        if counts[e] > nd:  # host-side overflow (exact fp32)
            xo = xr[idxs[e][nd:]]
            ho = np.maximum(xo @ w1[e].T + b1[e], 0.0)
            yo = ho @ w2[e].T + b2[e]
            out[idxs[e][nd:]] += wts[e][nd:, None] * yo
    return out.reshape(B, T, C)

